# revision 4
# baseline (speedup 1.0000x reference)
"""Trainium2 Bass kernel for nn_GTN4o_40132174414152 (2-layer graph transformer +
edge predictor). Self-contained: host-side sharding/index prep + SPMD Bass
kernel on 8 NeuronCores.

Strategy: attention rows (dst) sharded over 8 cores (512 rows each). The dense
(N,N) attention is effectively sparse (bias -1e9 off graph edges/diagonal), so
we compute it as an edge-list kernel: per-edge k/v rows fetched by indirect
DMA, per-edge scores via one-hot expansion matmuls on the tensor engine,
softmax without max-subtraction (scores are O(1)), and PV + softmax-denominator
accumulated with a single one-hot matmul per 128-edge tile. Node features and
params are replicated; only the (N,D) activations are all-gathered between
layers. Predictor pairs are data-parallel over cores.
"""

import numpy as np

import bass_rust as _bass_rust
import concourse.bass as bass
import concourse.tile as _tile_mod
import concourse.tile as tile
from concourse import mybir
from concourse.bass import IndirectOffsetOnAxis
from concourse.bass_utils import run_bass_kernel_spmd
from concourse.masks import make_identity
from concourse.vector_clock import ScopedClock as _ScopedClock

# ---------------------------------------------------------------------------
# Workarounds for the walrus build in this environment: at most ONE sync wait
# per instruction is supported.
# ---------------------------------------------------------------------------


def _drain_and_barrier_split(self, tick_clock, wait_clock):
    drain_inst = self.nc.sync.drain()
    wait_clock.add_sem_waits(
        drain_inst.ins, _ScopedClock({None: tick_clock.global_clock})
    )
    waits = list(drain_inst.ins.sync_info.on_wait)
    if len(waits) > 1:
        drain_inst.ins.sync_info.on_wait = waits[:1]
        for w in waits[1:]:
            d2 = self.nc.sync.drain()
            if d2.ins.sync_info is None:
                d2.ins.sync_info = _bass_rust.SyncInfo(on_wait=[w], on_update=[])
            else:
                d2.ins.sync_info.on_wait = [w]

    self.nc.all_engine_barrier()
    assert self.sems is not None
    popped = self.nc._tile_sem_poison_stack.pop()
    assert popped is self._sem_poison
    self.nc.clear_and_free_semaphores(list(self.sems.allocated().values()))
    self.nc.all_engine_barrier()


_tile_mod.TileContext._drain_and_barrier = _drain_and_barrier_split

_split_counter = [0]


def _split_multi_waits(nc):
    for f in nc.m.functions:
        for bb in f.blocks:
            insts = bb.instructions
            if not any(
                inst.sync_info and len(inst.sync_info.on_wait) > 1 for inst in insts
            ):
                continue
            out = []
            for inst in insts:
                si = inst.sync_info
                waits = list(si.on_wait) if si and si.on_wait else []
                if len(waits) > 1:
                    for w in waits[:-1]:
                        _split_counter[0] += 1
                        es = _bass_rust.InstEventSemaphore(
                            name=f"split_wait_{_split_counter[0]}", ins=[], outs=[]
                        )
                        es.engine = inst.engine
                        es.sync_info = _bass_rust.SyncInfo(on_wait=[w], on_update=[])
                        nc.register_instruction(es, overwrite=True)
                        out.append(es)
                    si.on_wait = [waits[-1]]
                out.append(inst)
            bb.instructions = out


# ---------------------------------------------------------------------------
# Problem constants (hardcoded per the contract)
# ---------------------------------------------------------------------------
N = 4096
D = 256
H = 8
DH = 32
IN_FEATS = 128
E = 131072
EP = 65536
NCORES = 8
P = 128
R = N // NCORES          # 512 dst rows per core
NDT = R // P             # 4 dst tiles per core
PB = EP // NCORES // P   # 64 predictor tiles per sign per core
NEG_SLOPE = 0.2
EPS = 1e-5
INV_SQRT_DH = 1.0 / np.sqrt(np.float32(DH))
PAD_BIAS = -1.0e30       # exp(x + PAD_BIAS) == 0 exactly in fp32

dt = mybir.dt

# fp32r (reduced-precision fast matmul) toggles for the one-hot selection
# matmuls. Off = exact fp32 (4 cyc/row).
QG_F32R = True
PV_F32R = True


# ---------------------------------------------------------------------------
# Host-side preprocessing
# ---------------------------------------------------------------------------


def _dedupe_edges(edge_index, edge_weight):
    """bias = full(-1e9); bias[diag]=0; bias[dst,src]=ew (in order, last wins).
    Returns (src, dst, w) for all finite-bias entries, sorted by (dst, src)."""
    src = np.asarray(edge_index[0]).astype(np.int64)
    dst = np.asarray(edge_index[1]).astype(np.int64)
    w = np.asarray(edge_weight).astype(np.float32)
    all_src = np.concatenate([np.arange(N, dtype=np.int64), src])
    all_dst = np.concatenate([np.arange(N, dtype=np.int64), dst])
    all_w = np.concatenate([np.zeros(N, dtype=np.float32), w])
    keys = all_dst * N + all_src
    rev = keys[::-1]
    _, idx_rev = np.unique(rev, return_index=True)
    keep = len(keys) - 1 - idx_rev  # last occurrence, ascending (dst, src)
    return all_src[keep], all_dst[keep], all_w[keep]


def _shard_edges(src, dst, w, n_et):
    """Per-core edge arrays, grouped by dst tile, padded to n_et tiles of 128
    edges per dst tile. Returns per-core dicts of (P, NDT*n_et) arrays."""
    out = []
    ncols = NDT * n_et
    for c in range(NCORES):
        src_a = np.zeros((P, ncols), dtype=np.int32)
        dl_a = np.zeros((P, ncols), dtype=np.float32)
        ew_a = np.full((P, ncols), PAD_BIAS, dtype=np.float32)
        for t in range(NDT):
            lo = c * R + t * P
            m = (dst >= lo) & (dst < lo + P)
            s_t, d_t, w_t = src[m], dst[m], w[m]
            cnt = len(s_t)
            assert cnt <= n_et * P, f"edge tile overflow: {cnt} > {n_et * P}"
            cols = np.arange(cnt) // P + t * n_et
            rows = np.arange(cnt) % P
            src_a[rows, cols] = s_t
            dl_a[rows, cols] = (d_t - lo).astype(np.float32)
            ew_a[rows, cols] = w_t
        out.append({"src": src_a, "dl": dl_a, "ew": ew_a})
    return out


def _max_edge_tiles(*edge_sets):
    n_et = 0
    for src, dst, w in edge_sets:
        for c in range(NCORES):
            for t in range(NDT):
                lo = c * R + t * P
                cnt = int(((dst >= lo) & (dst < lo + P)).sum())
                n_et = max(n_et, -(-cnt // P))
    return n_et


# ---------------------------------------------------------------------------
# Kernel program builder
# ---------------------------------------------------------------------------


def _build_program(n_et, b2_val):
    ET = n_et
    NT = NDT * ET  # edge-tile columns per core per layer

    nc = bass.Bass(num_swdge_queues=4)

    def din(name, shape, dty=dt.float32):
        return nc.dram_tensor(name, shape, dty, kind="ExternalInput")

    # --- inputs ---
    xT_d = din("xT", (IN_FEATS, N))
    ipwT_d = din("ipwT", (IN_FEATS, D))
    ipb_d = din("ipb", (P, D))
    wts = {}
    for l in range(2):
        wts[l] = {
            "qwT": din(f"l{l}_qwT", (D, D)),
            "kwT": din(f"l{l}_kwT", (D, D)),
            "vwT": din(f"l{l}_vwT", (D, D)),
            "owT": din(f"l{l}_owT", (D, D)),
            "fwT": din(f"l{l}_fwT", (D, D)),
            "qb": din(f"l{l}_qb", (P, D)),
            "kb": din(f"l{l}_kb", (P, D)),
            "vb": din(f"l{l}_vb", (P, D)),
            "ob": din(f"l{l}_ob", (P, D)),
            "fb": din(f"l{l}_fb", (P, D)),
            "n1g": din(f"l{l}_n1g", (P, D)),
            "n1b": din(f"l{l}_n1b", (P, D)),
            "n2g": din(f"l{l}_n2g", (P, D)),
            "n2b": din(f"l{l}_n2b", (P, D)),
            "src": din(f"l{l}_src", (P, NT), dt.int32),
            "dl": din(f"l{l}_dl", (P, NT)),
            "ew": din(f"l{l}_ew", (P, NT)),
        }
    w1T_d = din("w1T", (D, P))
    b1b_d = din("b1b", (P, P))
    w2b_d = din("w2b", (P, P))
    winids_d = din("winids", (P, NDT), dt.int32)
    pidx_d = {
        k: din(k, (P, PB), dt.int32) for k in ("psrc", "pdst", "nsrc", "ndst")
    }

    # --- internal DRAM ---
    h_proj_d = nc.dram_tensor("h_proj", (N, D), dt.float32)
    knat_d = [nc.dram_tensor(f"knat{l}", (N, D), dt.float32) for l in range(2)]
    vnat_d = [nc.dram_tensor(f"vnat{l}", (N, D), dt.float32) for l in range(2)]
    agin_d = [nc.dram_tensor(f"agin{l}", (R, D), dt.float32) for l in range(2)]
    agout_d = [
        nc.dram_tensor(f"agout{l}", (N, D), dt.float32, addr_space="Shared")
        for l in range(2)
    ]

    # --- outputs ---
    pos_out = nc.dram_tensor("pos_out", (EP // NCORES, 1), dt.float32,
                             kind="ExternalOutput")
    neg_out = nc.dram_tensor("neg_out", (EP // NCORES, 1), dt.float32,
                             kind="ExternalOutput")
    hcomb_out = nc.dram_tensor("hcomb_out", (N, D), dt.float32,
                               kind="ExternalOutput")

    f32, f32r, i32 = dt.float32, dt.float32r, dt.int32

    with tile.TileContext(nc) as tc:
        # ---------------- persistent pools ----------------
        with (
            tc.tile_pool(name="const", bufs=1) as cst,
            tc.tile_pool(name="wpool", bufs=1) as wp,
            tc.tile_pool(name="work", bufs=3) as wk,
            tc.tile_pool(name="gath", bufs=4) as gp,
            tc.tile_pool(name="small", bufs=4) as sp,
            tc.tile_pool(name="psA", bufs=3, space="PSUM") as psA,
            tc.tile_pool(name="psB", bufs=3, space="PSUM") as psB,
            tc.tile_pool(name="psO", bufs=1, space="PSUM") as psO,
        ):
            # constants
            iota_f = cst.tile([P, P], f32)
            nc.gpsimd.iota(iota_f[:], pattern=[[1, P]], base=0,
                           channel_multiplier=0,
                           allow_small_or_imprecise_dtypes=True)
            ident = cst.tile([P, P], f32)
            make_identity(nc, ident[:])
            identr = cst.tile([P, P], f32r)
            nc.scalar.copy(out=identr[:], in_=ident[:])

            xT_sb = cst.tile([IN_FEATS, N], f32)
            nc.sync.dma_start(out=xT_sb[:], in_=xT_d[:])
            ipwT_sb = cst.tile([IN_FEATS, D], f32)
            nc.sync.dma_start(out=ipwT_sb[:], in_=ipwT_d[:])
            ipb_sb = cst.tile([P, D], f32)
            nc.sync.dma_start(out=ipb_sb[:], in_=ipb_d[:])

            def load_w(name, dram, rows=D, cols=D):
                t0 = cst.tile([P, cols], f32, tag=name + "_0")
                nc.sync.dma_start(out=t0[:], in_=dram[0:P, :])
                if rows == D:
                    t1 = cst.tile([P, cols], f32, tag=name + "_1")
                    nc.sync.dma_start(out=t1[:], in_=dram[P:D, :])
                    return (t0, t1)
                return (t0,)

            W = {}
            for l in range(2):
                W[l] = {}
                for k in ("qwT", "kwT", "vwT", "owT", "fwT"):
                    W[l][k] = load_w(f"l{l}{k}", wts[l][k])
                for k in ("qb", "kb", "vb", "ob", "fb", "n1g", "n1b", "n2g",
                          "n2b"):
                    t = cst.tile([P, D], f32, tag=f"l{l}{k}")
                    nc.sync.dma_start(out=t[:], in_=wts[l][k][:])
                    W[l][k] = t
                for k in ("src", "dl", "ew"):
                    dty = i32 if k == "src" else f32
                    t = cst.tile([P, NT], dty, tag=f"l{l}{k}")
                    nc.sync.dma_start(out=t[:], in_=wts[l][k][:])
                    W[l][k] = t
            w1T_sb = load_w("w1T", w1T_d, cols=P)
            b1b_sb = cst.tile([P, P], f32)
            nc.sync.dma_start(out=b1b_sb[:], in_=b1b_d[:])
            w2b_sb = cst.tile([P, P], f32)
            nc.sync.dma_start(out=w2b_sb[:], in_=w2b_d[:])
            winids_sb = cst.tile([P, NDT], i32)
            nc.sync.dma_start(out=winids_sb[:], in_=winids_d[:])
            pidx_sb = {}
            for k, dtens in pidx_d.items():
                t = cst.tile([P, PB], i32, tag=k)
                nc.sync.dma_start(out=t[:], in_=dtens[:])
                pidx_sb[k] = t

            # -------- helpers --------
            def transpose_pair(x_sb):
                """x (128, 256) -> two SBUF tiles (128,128): xT halves."""
                outs = []
                for half in range(2):
                    tp = psA.tile([P, P], f32, tag="ps_tr")
                    nc.tensor.transpose(
                        out=tp[:], in_=x_sb[:, half * P:(half + 1) * P],
                        identity=ident[:],
                    )
                    cs = wk.tile([P, P], f32, tag="trsb")
                    nc.scalar.copy(out=cs[:], in_=tp[:])
                    outs.append(cs)
                return outs

            def nat_matmul(x_sb, wT, out_ps):
                """out_ps (128, cols) = x_sb (128,256) @ W.T, wT = host W.T tiles."""
                xt = transpose_pair(x_sb)
                nc.tensor.matmul(out=out_ps[:], lhsT=xt[0][:], rhs=wT[0][:],
                                 start=True, stop=False)
                nc.tensor.matmul(out=out_ps[:], lhsT=xt[1][:], rhs=wT[1][:],
                                 start=False, stop=True)

            def layer_norm(x_sb, g_sb, b_sb, out_sb):
                m = sp.tile([P, 1], f32, tag="ln_m")
                nc.vector.tensor_reduce(out=m[:], in_=x_sb[:],
                                        axis=mybir.AxisListType.X,
                                        op=mybir.AluOpType.add)
                negm = sp.tile([P, 1], f32, tag="ln_negm")
                nc.vector.tensor_scalar_mul(negm[:], m[:], -1.0 / D)
                xc = wk.tile([P, D], f32, tag="ln_xc")
                nc.vector.tensor_scalar_add(xc[:], x_sb[:], negm[:])
                sq = wk.tile([P, D], f32, tag="ln_sq")
                vr = sp.tile([P, 1], f32, tag="ln_vr")
                nc.scalar.activation(out=sq[:], in_=xc[:],
                                     func=mybir.ActivationFunctionType.Square,
                                     accum_out=vr[:])
                t1 = sp.tile([P, 1], f32, tag="ln_t1")
                nc.vector.tensor_scalar(out=t1[:], in0=vr[:], scalar1=1.0 / D,
                                        scalar2=EPS, op0=mybir.AluOpType.mult,
                                        op1=mybir.AluOpType.add)
                sd = sp.tile([P, 1], f32, tag="ln_sd")
                nc.scalar.activation(out=sd[:], in_=t1[:],
                                     func=mybir.ActivationFunctionType.Sqrt)
                rs = sp.tile([P, 1], f32, tag="ln_rs")
                nc.vector.reciprocal(out=rs[:], in_=sd[:])
                xn = wk.tile([P, D], f32, tag="ln_xn")
                nc.vector.tensor_scalar_mul(xn[:], xc[:], rs[:])
                nc.vector.tensor_tensor(out=out_sb[:], in0=xn[:], in1=g_sb[:],
                                        op=mybir.AluOpType.mult)
                nc.vector.tensor_tensor(out=out_sb[:], in0=out_sb[:],
                                        in1=b_sb[:], op=mybir.AluOpType.add)

            # ---------------- phase A: input projection ----------------
            for t in range(N // P):
                hp_ps = psB.tile([P, D], f32, tag="ps_mm")
                nc.tensor.matmul(out=hp_ps[:],
                                 lhsT=xT_sb[:, t * P:(t + 1) * P],
                                 rhs=ipwT_sb[:], start=True, stop=True)
                hp = wk.tile([P, D], f32, tag="hp")
                nc.vector.tensor_tensor(out=hp[:], in0=hp_ps[:], in1=ipb_sb[:],
                                        op=mybir.AluOpType.add)
                nc.sync.dma_start(out=h_proj_d[t * P:(t + 1) * P, :], in_=hp[:])

            # ---------------- per-layer ----------------
            for l in range(2):
                h_full = h_proj_d if l == 0 else agout_d[0]
                Wl = W[l]

                # B1: k,v tables for all rows
                for t in range(N // P):
                    h_t = wk.tile([P, D], f32, tag="kv_h")
                    nc.sync.dma_start(out=h_t[:],
                                      in_=h_full[t * P:(t + 1) * P, :])
                    ht = transpose_pair(h_t)
                    for nm, wkey, bkey, dest in (
                        ("k", "kwT", "kb", knat_d[l]),
                        ("v", "vwT", "vb", vnat_d[l]),
                    ):
                        ps = psB.tile([P, D], f32, tag="ps_mm")
                        nc.tensor.matmul(out=ps[:], lhsT=ht[0][:],
                                         rhs=Wl[wkey][0][:], start=True,
                                         stop=False)
                        nc.tensor.matmul(out=ps[:], lhsT=ht[1][:],
                                         rhs=Wl[wkey][1][:], start=False,
                                         stop=True)
                        o = wk.tile([P, D], f32, tag="kv_o")
                        nc.vector.tensor_tensor(out=o[:], in0=ps[:],
                                                in1=Wl[bkey][:],
                                                op=mybir.AluOpType.add)
                        nc.sync.dma_start(out=dest[t * P:(t + 1) * P, :],
                                          in_=o[:])

                # B2: attention + FFN per dst tile
                for dtile in range(NDT):
                    # window rows: h and q
                    h_win = wk.tile([P, D], f32, tag="h_win")
                    nc.gpsimd.indirect_dma_start(
                        out=h_win[:], out_offset=None, in_=h_full[:],
                        in_offset=IndirectOffsetOnAxis(
                            ap=winids_sb[:, dtile:dtile + 1], axis=0),
                    )
                    q_ps = psB.tile([P, D], f32, tag="ps_mm")
                    nat_matmul(h_win, Wl["qwT"], q_ps)
                    q_win = wk.tile([P, D], f32, tag="q_win")
                    nc.vector.tensor_tensor(out=q_win[:], in0=q_ps[:],
                                            in1=Wl["qb"][:],
                                            op=mybir.AluOpType.add)
                    if QG_F32R:
                        q_win_r = wk.tile([P, D], f32r, tag="q_win_r")
                        nc.scalar.copy(out=q_win_r[:], in_=q_win[:])
                        q_rhs = q_win_r
                    else:
                        q_rhs = q_win

                    o_acc = psO.tile([P, 33 * H], f32, tag="o_acc")
                    for et in range(ET):
                        col = dtile * ET + et
                        kg = gp.tile([P, D], f32, tag="kg")
                        nc.gpsimd.indirect_dma_start(
                            out=kg[:], out_offset=None, in_=knat_d[l][:],
                            in_offset=IndirectOffsetOnAxis(
                                ap=Wl["src"][:, col:col + 1], axis=0),
                        )
                        vg = gp.tile([P, D], f32, tag="vg")
                        nc.gpsimd.indirect_dma_start(
                            out=vg[:], out_offset=None, in_=vnat_d[l][:],
                            in_offset=IndirectOffsetOnAxis(
                                ap=Wl["src"][:, col:col + 1], axis=0),
                        )
                        # one-hot S (edges x dst) and its transpose
                        se_dt = f32r if PV_F32R else f32
                        se = wk.tile([P, P], se_dt, tag="se")
                        nc.vector.tensor_scalar(
                            out=se[:], in0=iota_f[:],
                            scalar1=Wl["dl"][:, col:col + 1], scalar2=None,
                            op0=mybir.AluOpType.is_equal,
                        )
                        st_dt = f32r if QG_F32R else f32
                        st_ps = psA.tile([P, P], st_dt, tag="ps_tr")
                        if QG_F32R and not PV_F32R:
                            ser = wk.tile([P, P], f32r, tag="ser")
                            nc.scalar.copy(out=ser[:], in_=se[:])
                            tr_in = ser
                        elif (not QG_F32R) and PV_F32R:
                            sef = wk.tile([P, P], f32, tag="sef")
                            nc.scalar.copy(out=sef[:], in_=se[:])
                            tr_in = sef if not QG_F32R else se
                        else:
                            tr_in = se
                        nc.tensor.transpose(
                            out=st_ps[:], in_=tr_in[:],
                            identity=(identr[:] if st_dt == f32r else ident[:]),
                        )
                        st = wk.tile([P, P], st_dt, tag="st")
                        nc.scalar.copy(out=st[:], in_=st_ps[:])
                        # qg = one-hot select of q rows per edge
                        qg_ps = psB.tile([P, D], f32, tag="ps_mm")
                        nc.tensor.matmul(out=qg_ps[:], lhsT=st[:], rhs=q_rhs[:],
                                         start=True, stop=True)
                        # per-edge per-head dot
                        prod = wk.tile([P, D], f32, tag="prod")
                        nc.vector.tensor_tensor(out=prod[:], in0=qg_ps[:],
                                                in1=kg[:],
                                                op=mybir.AluOpType.mult)
                        s8 = sp.tile([P, H], f32, tag="s8")
                        nc.vector.tensor_reduce(
                            out=s8[:],
                            in_=prod[:].rearrange("p (h c) -> p h c", h=H),
                            axis=mybir.AxisListType.X, op=mybir.AluOpType.add)
                        p8 = sp.tile([P, H], f32, tag="p8")
                        nc.scalar.activation(
                            out=p8[:], in_=s8[:],
                            func=mybir.ActivationFunctionType.Exp,
                            bias=Wl["ew"][:, col:col + 1],
                            scale=float(INV_SQRT_DH),
                        )
                        # pv_aug = [p*v | p] per head
                        pv_dt = f32r if PV_F32R else f32
                        pv = wk.tile([P, 33 * H], pv_dt, tag="pv")
                        pv_v = pv[:].rearrange("p (h c) -> p h c", c=33)
                        p8_b = p8[:].rearrange(
                            "p (h o) -> p h o", o=1).broadcast_to([P, H, DH])
                        nc.vector.tensor_tensor(
                            out=pv_v[:, :, 0:DH],
                            in0=vg[:].rearrange("p (h c) -> p h c", h=H),
                            in1=p8_b, op=mybir.AluOpType.mult)
                        nc.vector.tensor_copy(
                            out=pv_v[:, :, DH:DH + 1],
                            in_=p8[:].rearrange("p (h o) -> p h o", o=1))
                        nc.tensor.matmul(out=o_acc[:], lhsT=se[:], rhs=pv[:],
                                         start=(et == 0), stop=(et == ET - 1))

                    # normalize: o = num / den
                    ov = o_acc[:].rearrange("p (h c) -> p h c", c=33)
                    den = sp.tile([P, H], f32, tag="den")
                    nc.vector.tensor_copy(out=den[:], in_=ov[:, :, DH:DH + 1])
                    rden = sp.tile([P, H], f32, tag="rden")
                    nc.vector.reciprocal(out=rden[:], in_=den[:])
                    o_sb = wk.tile([P, D], f32, tag="o_sb")
                    rden_b = rden[:].rearrange(
                        "p (h o) -> p h o", o=1).broadcast_to([P, H, DH])
                    nc.vector.tensor_tensor(
                        out=o_sb[:].rearrange("p (h c) -> p h c", h=H),
                        in0=ov[:, :, 0:DH], in1=rden_b,
                        op=mybir.AluOpType.mult)

                    # out projection + residual + LN1
                    op_ps = psB.tile([P, D], f32, tag="ps_mm")
                    nat_matmul(o_sb, Wl["owT"], op_ps)
                    x1 = wk.tile([P, D], f32, tag="x1")
                    nc.vector.tensor_tensor(out=x1[:], in0=op_ps[:],
                                            in1=Wl["ob"][:],
                                            op=mybir.AluOpType.add)
                    nc.vector.tensor_tensor(out=x1[:], in0=x1[:], in1=h_win[:],
                                            op=mybir.AluOpType.add)
                    h1 = wk.tile([P, D], f32, tag="h1")
                    layer_norm(x1, Wl["n1g"], Wl["n1b"], h1)

                    # FF + residual + LN2
                    ff_ps = psB.tile([P, D], f32, tag="ps_mm")
                    nat_matmul(h1, Wl["fwT"], ff_ps)
                    x2 = wk.tile([P, D], f32, tag="x2")
                    nc.vector.tensor_tensor(out=x2[:], in0=ff_ps[:],
                                            in1=Wl["fb"][:],
                                            op=mybir.AluOpType.add)
                    nc.vector.tensor_tensor(out=x2[:], in0=x2[:], in1=h1[:],
                                            op=mybir.AluOpType.add)
                    h2 = wk.tile([P, D], f32, tag="h2")
                    layer_norm(x2, Wl["n2g"], Wl["n2b"], h2)
                    nc.sync.dma_start(
                        out=agin_d[l][dtile * P:(dtile + 1) * P, :], in_=h2[:])

                nc.gpsimd.collective_compute(
                    "AllGather", mybir.AluOpType.bypass,
                    ins=[agin_d[l][:]], outs=[agout_d[l][:]],
                    replica_groups=[list(range(NCORES))],
                )

            # ---------------- phase C: h_combined ----------------
            for t in range(N // P):
                a = wk.tile([P, D], f32, tag="hc_a")
                nc.sync.dma_start(out=a[:], in_=agout_d[1][t * P:(t + 1) * P, :])
                b = wk.tile([P, D], f32, tag="hc_b")
                nc.sync.dma_start(out=b[:], in_=h_proj_d[t * P:(t + 1) * P, :])
                hc = wk.tile([P, D], f32, tag="hc")
                nc.vector.tensor_tensor(out=hc[:], in0=a[:], in1=b[:],
                                        op=mybir.AluOpType.add)
                nc.sync.dma_start(out=hcomb_out[t * P:(t + 1) * P, :], in_=hc[:])

            # ---------------- phase D: predictor ----------------
            for sign, (skey, dkey, dest) in enumerate(
                (("psrc", "pdst", pos_out), ("nsrc", "ndst", neg_out))
            ):
                for b_i in range(PB):
                    hs = gp.tile([P, D], f32, tag="pr_hs")
                    nc.gpsimd.indirect_dma_start(
                        out=hs[:], out_offset=None, in_=hcomb_out[:],
                        in_offset=IndirectOffsetOnAxis(
                            ap=pidx_sb[skey][:, b_i:b_i + 1], axis=0),
                    )
                    hd = gp.tile([P, D], f32, tag="pr_hd")
                    nc.gpsimd.indirect_dma_start(
                        out=hd[:], out_offset=None, in_=hcomb_out[:],
                        in_offset=IndirectOffsetOnAxis(
                            ap=pidx_sb[dkey][:, b_i:b_i + 1], axis=0),
                    )
                    z = wk.tile([P, D], f32, tag="pr_z")
                    nc.vector.tensor_tensor(out=z[:], in0=hs[:], in1=hd[:],
                                            op=mybir.AluOpType.mult)
                    z1_ps = psB.tile([P, P], f32, tag="ps_mm")
                    nat_matmul(z, w1T_sb, z1_ps)
                    z1 = wk.tile([P, P], f32, tag="pr_z1s")
                    nc.vector.tensor_tensor(out=z1[:], in0=z1_ps[:],
                                            in1=b1b_sb[:],
                                            op=mybir.AluOpType.add)
                    t02 = wk.tile([P, P], f32, tag="pr_t02")
                    nc.scalar.mul(t02[:], z1[:], NEG_SLOPE)
                    lr = wk.tile([P, P], f32, tag="pr_lr")
                    nc.vector.tensor_tensor(out=lr[:], in0=z1[:], in1=t02[:],
                                            op=mybir.AluOpType.max)
                    prod2 = wk.tile([P, P], f32, tag="pr_prod2")
                    nc.vector.tensor_tensor(out=prod2[:], in0=lr[:],
                                            in1=w2b_sb[:],
                                            op=mybir.AluOpType.mult)
                    sc_raw = sp.tile([P, 1], f32, tag="pr_scr")
                    nc.vector.tensor_reduce(out=sc_raw[:], in_=prod2[:],
                                            axis=mybir.AxisListType.X,
                                            op=mybir.AluOpType.add)
                    sc = sp.tile([P, 1], f32, tag="pr_sc")
                    nc.vector.tensor_scalar_add(sc[:], sc_raw[:],
                                                float(b2_val))
                    nc.sync.dma_start(out=dest[b_i * P:(b_i + 1) * P, :],
                                      in_=sc[:])

    _split_multi_waits(nc)
    return nc


# ---------------------------------------------------------------------------
# Entry point
# ---------------------------------------------------------------------------

_cache = {}


def kernel(**inputs):
    ins = {k: np.asarray(v) for k, v in inputs.items()}

    e0 = _dedupe_edges(ins["edge_index0"], ins["edge_weight0"])
    e1 = _dedupe_edges(ins["edge_index1"], ins["edge_weight1"])
    n_et = _max_edge_tiles(e0, e1)
    shards = [_shard_edges(*e0, n_et), _shard_edges(*e1, n_et)]

    b2_val = float(np.asarray(ins["pred_b2"]).reshape(-1)[0])

    key = (n_et, b2_val)
    if key not in _cache:
        _cache[key] = _build_program(n_et, b2_val)
    nc = _cache[key]

    def bcast(v, cols=D):
        v = np.asarray(v, np.float32).reshape(1, -1)
        return np.broadcast_to(v, (P, v.shape[1])).copy()

    common = {
        "xT": np.ascontiguousarray(ins["x"].T.astype(np.float32)),
        "ipwT": np.ascontiguousarray(
            ins["input_proj_w"].astype(np.float32).T),
        "ipb": bcast(ins["input_proj_b"]),
        "w1T": np.ascontiguousarray(ins["pred_w1"].astype(np.float32).T),
        "b1b": bcast(ins["pred_b1"]),
        "w2b": np.broadcast_to(
            ins["pred_w2"].astype(np.float32).reshape(1, P), (P, P)).copy(),
    }
    for l in range(2):
        in_w = ins[f"l{l}_in_w"].astype(np.float32)
        in_b = ins[f"l{l}_in_b"].astype(np.float32)
        common[f"l{l}_qwT"] = np.ascontiguousarray(in_w[0:D].T)
        common[f"l{l}_kwT"] = np.ascontiguousarray(in_w[D:2 * D].T)
        common[f"l{l}_vwT"] = np.ascontiguousarray(in_w[2 * D:3 * D].T)
        common[f"l{l}_owT"] = np.ascontiguousarray(
            ins[f"l{l}_out_w"].astype(np.float32).T)
        common[f"l{l}_fwT"] = np.ascontiguousarray(
            ins[f"l{l}_ff_w"].astype(np.float32).T)
        common[f"l{l}_qb"] = bcast(in_b[0:D])
        common[f"l{l}_kb"] = bcast(in_b[D:2 * D])
        common[f"l{l}_vb"] = bcast(in_b[2 * D:3 * D])
        common[f"l{l}_ob"] = bcast(ins[f"l{l}_out_b"])
        common[f"l{l}_fb"] = bcast(ins[f"l{l}_ff_b"])
        common[f"l{l}_n1g"] = bcast(ins[f"l{l}_n1_g"])
        common[f"l{l}_n1b"] = bcast(ins[f"l{l}_n1_b"])
        common[f"l{l}_n2g"] = bcast(ins[f"l{l}_n2_g"])
        common[f"l{l}_n2b"] = bcast(ins[f"l{l}_n2_b"])

    def tile_idx(arr_1d, c):
        a = np.asarray(arr_1d).astype(np.int32)[c * (EP // NCORES):(c + 1) * (EP // NCORES)]
        return np.ascontiguousarray(a.reshape(PB, P).T)

    in_maps = []
    for c in range(NCORES):
        m = dict(common)
        for l in range(2):
            sh = shards[l][c]
            m[f"l{l}_src"] = sh["src"]
            m[f"l{l}_dl"] = sh["dl"]
            m[f"l{l}_ew"] = sh["ew"]
        wi = np.empty((P, NDT), np.int32)
        for t in range(NDT):
            wi[:, t] = c * R + t * P + np.arange(P)
        m["winids"] = wi
        m["psrc"] = tile_idx(ins["pos_src"], c)
        m["pdst"] = tile_idx(ins["pos_dst"], c)
        m["nsrc"] = tile_idx(ins["neg_src"], c)
        m["ndst"] = tile_idx(ins["neg_dst"], c)
        in_maps.append(m)

    res = run_bass_kernel_spmd(nc, in_maps, core_ids=list(range(NCORES)))

    pos = np.concatenate([res.results[c]["pos_out"] for c in range(NCORES)], 0)
    neg = np.concatenate([res.results[c]["neg_out"] for c in range(NCORES)], 0)
    h_comb = res.results[0]["hcomb_out"]
    return pos, neg, h_comb


# revision 10
# speedup vs baseline: 1.2387x; 1.2387x over previous
"""Trainium2 Bass kernel for nn_GTN4o_40132174414152 (2-layer graph transformer +
edge predictor). Self-contained: host-side sharding/index prep + SPMD Bass
kernel on 8 NeuronCores.

Strategy: attention rows (dst) sharded over 8 cores (512 rows each). The dense
(N,N) attention is effectively sparse (bias -1e9 off graph edges/diagonal), so
we compute it as an edge-list kernel: per-edge k/v rows fetched by indirect
DMA, per-edge scores via one-hot expansion matmuls on the tensor engine,
softmax without max-subtraction (scores are O(1)), and PV + softmax-denominator
accumulated with a single one-hot matmul per 128-edge tile. Node features and
params are replicated; only the (N,D) activations are all-gathered between
layers. Predictor pairs are data-parallel over cores.
"""

import numpy as np

import bass_rust as _bass_rust
import concourse.bass as bass
import concourse.tile as _tile_mod
import concourse.tile as tile
from concourse import mybir
from concourse.bass import IndirectOffsetOnAxis
from concourse.bass_utils import run_bass_kernel_spmd
from concourse.masks import make_identity
from concourse.vector_clock import ScopedClock as _ScopedClock

# ---------------------------------------------------------------------------
# Workarounds for the walrus build in this environment: at most ONE sync wait
# per instruction is supported.
# ---------------------------------------------------------------------------


def _drain_and_barrier_split(self, tick_clock, wait_clock):
    drain_inst = self.nc.sync.drain()
    wait_clock.add_sem_waits(
        drain_inst.ins, _ScopedClock({None: tick_clock.global_clock})
    )
    waits = list(drain_inst.ins.sync_info.on_wait)
    if len(waits) > 1:
        drain_inst.ins.sync_info.on_wait = waits[:1]
        for w in waits[1:]:
            d2 = self.nc.sync.drain()
            if d2.ins.sync_info is None:
                d2.ins.sync_info = _bass_rust.SyncInfo(on_wait=[w], on_update=[])
            else:
                d2.ins.sync_info.on_wait = [w]

    self.nc.all_engine_barrier()
    assert self.sems is not None
    popped = self.nc._tile_sem_poison_stack.pop()
    assert popped is self._sem_poison
    self.nc.clear_and_free_semaphores(list(self.sems.allocated().values()))
    self.nc.all_engine_barrier()


_tile_mod.TileContext._drain_and_barrier = _drain_and_barrier_split

_split_counter = [0]


def _split_multi_waits(nc):
    for f in nc.m.functions:
        for bb in f.blocks:
            insts = bb.instructions
            if not any(
                inst.sync_info and len(inst.sync_info.on_wait) > 1 for inst in insts
            ):
                continue
            out = []
            for inst in insts:
                si = inst.sync_info
                waits = list(si.on_wait) if si and si.on_wait else []
                if len(waits) > 1:
                    for w in waits[:-1]:
                        _split_counter[0] += 1
                        es = _bass_rust.InstEventSemaphore(
                            name=f"split_wait_{_split_counter[0]}", ins=[], outs=[]
                        )
                        es.engine = inst.engine
                        es.sync_info = _bass_rust.SyncInfo(on_wait=[w], on_update=[])
                        nc.register_instruction(es, overwrite=True)
                        out.append(es)
                    si.on_wait = [waits[-1]]
                out.append(inst)
            bb.instructions = out


# ---------------------------------------------------------------------------
# Problem constants (hardcoded per the contract)
# ---------------------------------------------------------------------------
N = 4096
D = 256
H = 8
DH = 32
IN_FEATS = 128
E = 131072
EP = 65536
NCORES = 8
P = 128
R = N // NCORES          # 512 dst rows per core
NDT = R // P             # 4 dst tiles per core
PB = EP // NCORES // P   # 64 predictor tiles per sign per core
NEG_SLOPE = 0.2
EPS = 1e-5
INV_SQRT_DH = 1.0 / np.sqrt(np.float32(DH))
PAD_BIAS = -1.0e30       # exp(x + PAD_BIAS) == 0 exactly in fp32

dt = mybir.dt

# fp32r (reduced-precision fast matmul) toggles for the one-hot selection
# matmuls. Off = exact fp32 (4 cyc/row).
QG_F32R = True
PV_F32R = True
KV_F32R = True   # k/v/q projection matmuls in fp32r
GG = 1           # edge tiles per indirect-DMA gather call
BG = 4           # node tiles per batched regular DMA
PG = 2           # predictor tiles per group


# ---------------------------------------------------------------------------
# Host-side preprocessing
# ---------------------------------------------------------------------------


def _dedupe_edges(edge_index, edge_weight):
    """bias = full(-1e9); bias[diag]=0; bias[dst,src]=ew (in order, last wins).
    Returns (src, dst, w) for all finite-bias entries, sorted by (dst, src)."""
    src = np.asarray(edge_index[0]).astype(np.int64)
    dst = np.asarray(edge_index[1]).astype(np.int64)
    w = np.asarray(edge_weight).astype(np.float32)
    all_src = np.concatenate([np.arange(N, dtype=np.int64), src])
    all_dst = np.concatenate([np.arange(N, dtype=np.int64), dst])
    all_w = np.concatenate([np.zeros(N, dtype=np.float32), w])
    keys = all_dst * N + all_src
    rev = keys[::-1]
    _, idx_rev = np.unique(rev, return_index=True)
    keep = len(keys) - 1 - idx_rev  # last occurrence, ascending (dst, src)
    return all_src[keep], all_dst[keep], all_w[keep]


def _shard_edges(src, dst, w, n_et):
    """Per-core edge arrays, grouped by dst tile, padded to n_et tiles of 128
    edges per dst tile. Returns per-core dicts of (P, NDT*n_et) arrays."""
    out = []
    ncols = NDT * n_et
    for c in range(NCORES):
        src_a = np.zeros((P, ncols), dtype=np.int32)
        dl_a = np.zeros((P, ncols), dtype=np.float32)
        ew_a = np.full((P, ncols), PAD_BIAS, dtype=np.float32)
        for t in range(NDT):
            lo = c * R + t * P
            m = (dst >= lo) & (dst < lo + P)
            s_t, d_t, w_t = src[m], dst[m], w[m]
            cnt = len(s_t)
            assert cnt <= n_et * P, f"edge tile overflow: {cnt} > {n_et * P}"
            cols = np.arange(cnt) // P + t * n_et
            rows = np.arange(cnt) % P
            src_a[rows, cols] = s_t
            dl_a[rows, cols] = (d_t - lo).astype(np.float32)
            ew_a[rows, cols] = w_t
        out.append({"src": src_a, "dl": dl_a, "ew": ew_a})
    return out


def _max_edge_tiles(*edge_sets):
    n_et = 0
    for src, dst, w in edge_sets:
        for c in range(NCORES):
            for t in range(NDT):
                lo = c * R + t * P
                cnt = int(((dst >= lo) & (dst < lo + P)).sum())
                n_et = max(n_et, -(-cnt // P))
    return n_et


# ---------------------------------------------------------------------------
# Kernel program builder
# ---------------------------------------------------------------------------


def _build_program(n_et, b2_val):
    ET = n_et
    NT = NDT * ET  # edge-tile columns per core per layer

    nc = bass.Bass(num_swdge_queues=4)

    def din(name, shape, dty=dt.float32):
        return nc.dram_tensor(name, shape, dty, kind="ExternalInput")

    # --- inputs ---
    xT_d = din("xT", (IN_FEATS, N))
    ipwT_d = din("ipwT", (IN_FEATS, D))
    ipb_d = din("ipb", (P, D))
    wts = {}
    for l in range(2):
        wts[l] = {
            "qwT": din(f"l{l}_qwT", (D, D)),
            "kwT": din(f"l{l}_kwT", (D, D)),
            "vwT": din(f"l{l}_vwT", (D, D)),
            "owT": din(f"l{l}_owT", (D, D)),
            "fwT": din(f"l{l}_fwT", (D, D)),
            "qb": din(f"l{l}_qb", (P, D)),
            "kb": din(f"l{l}_kb", (P, D)),
            "vb": din(f"l{l}_vb", (P, D)),
            "ob": din(f"l{l}_ob", (P, D)),
            "fb": din(f"l{l}_fb", (P, D)),
            "n1g": din(f"l{l}_n1g", (P, D)),
            "n1b": din(f"l{l}_n1b", (P, D)),
            "n2g": din(f"l{l}_n2g", (P, D)),
            "n2b": din(f"l{l}_n2b", (P, D)),
            "src": din(f"l{l}_src", (P, NT), dt.int32),
            "dl": din(f"l{l}_dl", (P, NT)),
            "ew": din(f"l{l}_ew", (P, NT)),
        }
    w1T_d = din("w1T", (D, P))
    b1b_d = din("b1b", (P, P))
    w2b_d = din("w2b", (P, P))
    winids_d = din("winids", (P, NDT), dt.int32)
    pidx_d = {
        k: din(k, (P, PB), dt.int32) for k in ("psrc", "pdst", "nsrc", "ndst")
    }

    # --- internal DRAM ---
    h_proj_d = nc.dram_tensor("h_proj", (N, D), dt.float32)
    knat_d = [nc.dram_tensor(f"knat{l}", (N, D), dt.float32) for l in range(2)]
    vnat_d = [nc.dram_tensor(f"vnat{l}", (N, D), dt.float32) for l in range(2)]
    agin_d = [nc.dram_tensor(f"agin{l}", (R, D), dt.float32) for l in range(2)]
    agout_d = [
        nc.dram_tensor(f"agout{l}", (N, D), dt.float32, addr_space="Shared")
        for l in range(2)
    ]

    # --- outputs ---
    pos_out = nc.dram_tensor("pos_out", (EP // NCORES, 1), dt.float32,
                             kind="ExternalOutput")
    neg_out = nc.dram_tensor("neg_out", (EP // NCORES, 1), dt.float32,
                             kind="ExternalOutput")
    hcomb_out = nc.dram_tensor("hcomb_out", (N, D), dt.float32,
                               kind="ExternalOutput")

    f32, f32r, i32 = dt.float32, dt.float32r, dt.int32

    with tile.TileContext(nc) as tc:
        # ---------------- persistent pools ----------------
        with (
            tc.tile_pool(name="const", bufs=1) as cst,
            tc.tile_pool(name="wpool", bufs=1) as wp,
            tc.tile_pool(name="work", bufs=3) as wk,
            tc.tile_pool(name="gath", bufs=8) as gp,
            tc.tile_pool(name="small", bufs=4) as sp,
            tc.tile_pool(name="psA", bufs=3, space="PSUM") as psA,
            tc.tile_pool(name="psB", bufs=3, space="PSUM") as psB,
            tc.tile_pool(name="psO", bufs=1, space="PSUM") as psO,
        ):
            # constants
            iota_f = cst.tile([P, P], f32)
            nc.gpsimd.iota(iota_f[:], pattern=[[1, P]], base=0,
                           channel_multiplier=0,
                           allow_small_or_imprecise_dtypes=True)
            ident = cst.tile([P, P], f32)
            make_identity(nc, ident[:])
            identr = cst.tile([P, P], f32r)
            nc.scalar.copy(out=identr[:], in_=ident[:])

            xT_sb = cst.tile([IN_FEATS, N], f32)
            nc.sync.dma_start(out=xT_sb[:], in_=xT_d[:])
            ipwT_sb = cst.tile([IN_FEATS, D], f32)
            nc.sync.dma_start(out=ipwT_sb[:], in_=ipwT_d[:])
            ipb_sb = cst.tile([P, D], f32)
            nc.sync.dma_start(out=ipb_sb[:], in_=ipb_d[:])

            def load_w(name, dram, rows=D, cols=D):
                t0 = cst.tile([P, cols], f32, tag=name + "_0")
                nc.sync.dma_start(out=t0[:], in_=dram[0:P, :])
                if rows == D:
                    t1 = cst.tile([P, cols], f32, tag=name + "_1")
                    nc.sync.dma_start(out=t1[:], in_=dram[P:D, :])
                    return (t0, t1)
                return (t0,)

            W = {}
            for l in range(2):
                W[l] = {}
                for k in ("qwT", "kwT", "vwT", "owT", "fwT"):
                    W[l][k] = load_w(f"l{l}{k}", wts[l][k])
                for k in ("qb", "kb", "vb", "ob", "fb", "n1g", "n1b", "n2g",
                          "n2b"):
                    t = cst.tile([P, D], f32, tag=f"l{l}{k}")
                    nc.sync.dma_start(out=t[:], in_=wts[l][k][:])
                    W[l][k] = t
                for k in ("src", "dl", "ew"):
                    dty = i32 if k == "src" else f32
                    t = cst.tile([P, NT], dty, tag=f"l{l}{k}")
                    nc.sync.dma_start(out=t[:], in_=wts[l][k][:])
                    W[l][k] = t
            w1T_sb = load_w("w1T", w1T_d, cols=P)
            b1b_sb = cst.tile([P, P], f32)
            nc.sync.dma_start(out=b1b_sb[:], in_=b1b_d[:])
            w2b_sb = cst.tile([P, P], f32)
            nc.sync.dma_start(out=w2b_sb[:], in_=w2b_d[:])
            winids_sb = cst.tile([P, NDT], i32)
            nc.sync.dma_start(out=winids_sb[:], in_=winids_d[:])
            pidx_sb = {}
            for k, dtens in pidx_d.items():
                t = cst.tile([P, PB], i32, tag=k)
                nc.sync.dma_start(out=t[:], in_=dtens[:])
                pidx_sb[k] = t

            # -------- helpers --------
            def transpose_pair(x_sb):
                """x (128, 256) -> two SBUF tiles (128,128): xT halves."""
                outs = []
                for half in range(2):
                    tp = psA.tile([P, P], f32, tag="ps_tr")
                    nc.tensor.transpose(
                        out=tp[:], in_=x_sb[:, half * P:(half + 1) * P],
                        identity=ident[:],
                    )
                    cs = wk.tile([P, P], f32, tag="trsb")
                    nc.scalar.copy(out=cs[:], in_=tp[:])
                    outs.append(cs)
                return outs

            def nat_matmul(x_sb, wT, out_ps):
                """out_ps (128, cols) = x_sb (128,256) @ W.T, wT = host W.T tiles."""
                xt = transpose_pair(x_sb)
                nc.tensor.matmul(out=out_ps[:], lhsT=xt[0][:], rhs=wT[0][:],
                                 start=True, stop=False)
                nc.tensor.matmul(out=out_ps[:], lhsT=xt[1][:], rhs=wT[1][:],
                                 start=False, stop=True)

            def layer_norm(x_sb, g_sb, b_sb, out_sb):
                m = sp.tile([P, 1], f32, tag="ln_m")
                nc.vector.tensor_reduce(out=m[:], in_=x_sb[:],
                                        axis=mybir.AxisListType.X,
                                        op=mybir.AluOpType.add)
                negm = sp.tile([P, 1], f32, tag="ln_negm")
                nc.vector.tensor_scalar_mul(negm[:], m[:], -1.0 / D)
                xc = wk.tile([P, D], f32, tag="ln_xc")
                nc.vector.tensor_scalar_add(xc[:], x_sb[:], negm[:])
                sq = wk.tile([P, D], f32, tag="ln_sq")
                vr = sp.tile([P, 1], f32, tag="ln_vr")
                nc.scalar.activation(out=sq[:], in_=xc[:],
                                     func=mybir.ActivationFunctionType.Square,
                                     accum_out=vr[:])
                t1 = sp.tile([P, 1], f32, tag="ln_t1")
                nc.vector.tensor_scalar(out=t1[:], in0=vr[:], scalar1=1.0 / D,
                                        scalar2=EPS, op0=mybir.AluOpType.mult,
                                        op1=mybir.AluOpType.add)
                sd = sp.tile([P, 1], f32, tag="ln_sd")
                nc.scalar.activation(out=sd[:], in_=t1[:],
                                     func=mybir.ActivationFunctionType.Sqrt)
                rs = sp.tile([P, 1], f32, tag="ln_rs")
                nc.vector.reciprocal(out=rs[:], in_=sd[:])
                xn = wk.tile([P, D], f32, tag="ln_xn")
                nc.vector.tensor_scalar_mul(xn[:], xc[:], rs[:])
                nc.vector.tensor_tensor(out=out_sb[:], in0=xn[:], in1=g_sb[:],
                                        op=mybir.AluOpType.mult)
                nc.vector.tensor_tensor(out=out_sb[:], in0=out_sb[:],
                                        in1=b_sb[:], op=mybir.AluOpType.add)

            # ---------------- phase A: input projection ----------------
            for t in range(N // P):
                hp_ps = psB.tile([P, D], f32, tag="ps_mm")
                nc.tensor.matmul(out=hp_ps[:],
                                 lhsT=xT_sb[:, t * P:(t + 1) * P],
                                 rhs=ipwT_sb[:], start=True, stop=True)
                hp = wk.tile([P, D], f32, tag="hp")
                nc.vector.tensor_tensor(out=hp[:], in0=hp_ps[:], in1=ipb_sb[:],
                                        op=mybir.AluOpType.add)
                nc.sync.dma_start(out=h_proj_d[t * P:(t + 1) * P, :], in_=hp[:])

            # ---------------- per-layer ----------------
            for l in range(2):
                h_full = h_proj_d if l == 0 else agout_d[0]
                Wl = W[l]

                # B1: k,v tables for all rows
                for t in range(N // P):
                    h_t = wk.tile([P, D], f32, tag="kv_h")
                    nc.sync.dma_start(out=h_t[:],
                                      in_=h_full[t * P:(t + 1) * P, :])
                    ht = transpose_pair(h_t)
                    for nm, wkey, bkey, dest in (
                        ("k", "kwT", "kb", knat_d[l]),
                        ("v", "vwT", "vb", vnat_d[l]),
                    ):
                        ps = psB.tile([P, D], f32, tag="ps_mm")
                        nc.tensor.matmul(out=ps[:], lhsT=ht[0][:],
                                         rhs=Wl[wkey][0][:], start=True,
                                         stop=False)
                        nc.tensor.matmul(out=ps[:], lhsT=ht[1][:],
                                         rhs=Wl[wkey][1][:], start=False,
                                         stop=True)
                        o = wk.tile([P, D], f32, tag="kv_o")
                        nc.vector.tensor_tensor(out=o[:], in0=ps[:],
                                                in1=Wl[bkey][:],
                                                op=mybir.AluOpType.add)
                        nc.sync.dma_start(out=dest[t * P:(t + 1) * P, :],
                                          in_=o[:])

                # B2: attention + FFN per dst tile
                for dtile in range(NDT):
                    # window rows: h and q
                    h_win = wk.tile([P, D], f32, tag="h_win")
                    nc.gpsimd.indirect_dma_start(
                        out=h_win[:], out_offset=None, in_=h_full[:],
                        in_offset=IndirectOffsetOnAxis(
                            ap=winids_sb[:, dtile:dtile + 1], axis=0),
                    )
                    q_ps = psB.tile([P, D], f32, tag="ps_mm")
                    nat_matmul(h_win, Wl["qwT"], q_ps)
                    q_win = wk.tile([P, D], f32, tag="q_win")
                    nc.vector.tensor_tensor(out=q_win[:], in0=q_ps[:],
                                            in1=Wl["qb"][:],
                                            op=mybir.AluOpType.add)
                    if QG_F32R:
                        q_win_r = wk.tile([P, D], f32r, tag="q_win_r")
                        nc.scalar.copy(out=q_win_r[:], in_=q_win[:])
                        q_rhs = q_win_r
                    else:
                        q_rhs = q_win

                    o_acc = psO.tile([P, 33 * H], f32, tag="o_acc")
                    for et in range(ET):
                        col = dtile * ET + et
                        kg = gp.tile([P, D], f32, tag="kg")
                        nc.gpsimd.indirect_dma_start(
                            out=kg[:], out_offset=None, in_=knat_d[l][:],
                            in_offset=IndirectOffsetOnAxis(
                                ap=Wl["src"][:, col:col + 1], axis=0),
                        )
                        vg = gp.tile([P, D], f32, tag="vg")
                        nc.gpsimd.indirect_dma_start(
                            out=vg[:], out_offset=None, in_=vnat_d[l][:],
                            in_offset=IndirectOffsetOnAxis(
                                ap=Wl["src"][:, col:col + 1], axis=0),
                        )
                        # one-hot S (edges x dst) and its transpose
                        se_dt = f32r if PV_F32R else f32
                        se = wk.tile([P, P], se_dt, tag="se")
                        nc.vector.tensor_scalar(
                            out=se[:], in0=iota_f[:],
                            scalar1=Wl["dl"][:, col:col + 1], scalar2=None,
                            op0=mybir.AluOpType.is_equal,
                        )
                        st_dt = f32r if QG_F32R else f32
                        st_ps = psA.tile([P, P], st_dt, tag="ps_tr")
                        if QG_F32R and not PV_F32R:
                            ser = wk.tile([P, P], f32r, tag="ser")
                            nc.scalar.copy(out=ser[:], in_=se[:])
                            tr_in = ser
                        elif (not QG_F32R) and PV_F32R:
                            sef = wk.tile([P, P], f32, tag="sef")
                            nc.scalar.copy(out=sef[:], in_=se[:])
                            tr_in = sef if not QG_F32R else se
                        else:
                            tr_in = se
                        nc.tensor.transpose(
                            out=st_ps[:], in_=tr_in[:],
                            identity=(identr[:] if st_dt == f32r else ident[:]),
                        )
                        st = wk.tile([P, P], st_dt, tag="st")
                        nc.scalar.copy(out=st[:], in_=st_ps[:])
                        # qg = one-hot select of q rows per edge
                        qg_ps = psB.tile([P, D], f32, tag="ps_mm")
                        nc.tensor.matmul(out=qg_ps[:], lhsT=st[:], rhs=q_rhs[:],
                                         start=True, stop=True)
                        # per-edge per-head dot
                        prod = wk.tile([P, D], f32, tag="prod")
                        nc.vector.tensor_tensor(out=prod[:], in0=qg_ps[:],
                                                in1=kg[:],
                                                op=mybir.AluOpType.mult)
                        s8 = sp.tile([P, H], f32, tag="s8")
                        nc.vector.tensor_reduce(
                            out=s8[:],
                            in_=prod[:].rearrange("p (h c) -> p h c", h=H),
                            axis=mybir.AxisListType.X, op=mybir.AluOpType.add)
                        p8 = sp.tile([P, H], f32, tag="p8")
                        nc.scalar.activation(
                            out=p8[:], in_=s8[:],
                            func=mybir.ActivationFunctionType.Exp,
                            bias=Wl["ew"][:, col:col + 1],
                            scale=float(INV_SQRT_DH),
                        )
                        # pv_aug = [p*v | p] per head
                        pv_dt = f32r if PV_F32R else f32
                        pv = wk.tile([P, 33 * H], pv_dt, tag="pv")
                        pv_v = pv[:].rearrange("p (h c) -> p h c", c=33)
                        p8_b = p8[:].rearrange(
                            "p (h o) -> p h o", o=1).broadcast_to([P, H, DH])
                        nc.vector.tensor_tensor(
                            out=pv_v[:, :, 0:DH],
                            in0=vg[:].rearrange("p (h c) -> p h c", h=H),
                            in1=p8_b, op=mybir.AluOpType.mult)
                        nc.vector.tensor_copy(
                            out=pv_v[:, :, DH:DH + 1],
                            in_=p8[:].rearrange("p (h o) -> p h o", o=1))
                        nc.tensor.matmul(out=o_acc[:], lhsT=se[:], rhs=pv[:],
                                         start=(et == 0), stop=(et == ET - 1))

                    # normalize: o = num / den
                    ov = o_acc[:].rearrange("p (h c) -> p h c", c=33)
                    den = sp.tile([P, H], f32, tag="den")
                    nc.vector.tensor_copy(out=den[:], in_=ov[:, :, DH:DH + 1])
                    rden = sp.tile([P, H], f32, tag="rden")
                    nc.vector.reciprocal(out=rden[:], in_=den[:])
                    o_sb = wk.tile([P, D], f32, tag="o_sb")
                    rden_b = rden[:].rearrange(
                        "p (h o) -> p h o", o=1).broadcast_to([P, H, DH])
                    nc.vector.tensor_tensor(
                        out=o_sb[:].rearrange("p (h c) -> p h c", h=H),
                        in0=ov[:, :, 0:DH], in1=rden_b,
                        op=mybir.AluOpType.mult)

                    # out projection + residual + LN1
                    op_ps = psB.tile([P, D], f32, tag="ps_mm")
                    nat_matmul(o_sb, Wl["owT"], op_ps)
                    x1 = wk.tile([P, D], f32, tag="x1")
                    nc.vector.tensor_tensor(out=x1[:], in0=op_ps[:],
                                            in1=Wl["ob"][:],
                                            op=mybir.AluOpType.add)
                    nc.vector.tensor_tensor(out=x1[:], in0=x1[:], in1=h_win[:],
                                            op=mybir.AluOpType.add)
                    h1 = wk.tile([P, D], f32, tag="h1")
                    layer_norm(x1, Wl["n1g"], Wl["n1b"], h1)

                    # FF + residual + LN2
                    ff_ps = psB.tile([P, D], f32, tag="ps_mm")
                    nat_matmul(h1, Wl["fwT"], ff_ps)
                    x2 = wk.tile([P, D], f32, tag="x2")
                    nc.vector.tensor_tensor(out=x2[:], in0=ff_ps[:],
                                            in1=Wl["fb"][:],
                                            op=mybir.AluOpType.add)
                    nc.vector.tensor_tensor(out=x2[:], in0=x2[:], in1=h1[:],
                                            op=mybir.AluOpType.add)
                    h2 = wk.tile([P, D], f32, tag="h2")
                    layer_norm(x2, Wl["n2g"], Wl["n2b"], h2)
                    nc.sync.dma_start(
                        out=agin_d[l][dtile * P:(dtile + 1) * P, :], in_=h2[:])

                nc.gpsimd.collective_compute(
                    "AllGather", mybir.AluOpType.bypass,
                    ins=[agin_d[l][:]], outs=[agout_d[l][:]],
                    replica_groups=[list(range(NCORES))],
                )

            # ---------------- phase C: h_combined ----------------
            for t in range(N // P):
                a = wk.tile([P, D], f32, tag="hc_a")
                nc.sync.dma_start(out=a[:], in_=agout_d[1][t * P:(t + 1) * P, :])
                b = wk.tile([P, D], f32, tag="hc_b")
                nc.sync.dma_start(out=b[:], in_=h_proj_d[t * P:(t + 1) * P, :])
                hc = wk.tile([P, D], f32, tag="hc")
                nc.vector.tensor_tensor(out=hc[:], in0=a[:], in1=b[:],
                                        op=mybir.AluOpType.add)
                nc.sync.dma_start(out=hcomb_out[t * P:(t + 1) * P, :], in_=hc[:])

            # ---------------- phase D: predictor ----------------
            for sign, (skey, dkey, dest) in enumerate(
                (("psrc", "pdst", pos_out), ("nsrc", "ndst", neg_out))
            ):
                for b_i in range(PB):
                    hs = gp.tile([P, D], f32, tag="pr_hs")
                    nc.gpsimd.indirect_dma_start(
                        out=hs[:], out_offset=None, in_=hcomb_out[:],
                        in_offset=IndirectOffsetOnAxis(
                            ap=pidx_sb[skey][:, b_i:b_i + 1], axis=0),
                    )
                    hd = gp.tile([P, D], f32, tag="pr_hd")
                    nc.gpsimd.indirect_dma_start(
                        out=hd[:], out_offset=None, in_=hcomb_out[:],
                        in_offset=IndirectOffsetOnAxis(
                            ap=pidx_sb[dkey][:, b_i:b_i + 1], axis=0),
                    )
                    z = wk.tile([P, D], f32, tag="pr_z")
                    nc.vector.tensor_tensor(out=z[:], in0=hs[:], in1=hd[:],
                                            op=mybir.AluOpType.mult)
                    z1_ps = psB.tile([P, P], f32, tag="ps_mm")
                    nat_matmul(z, w1T_sb, z1_ps)
                    z1 = wk.tile([P, P], f32, tag="pr_z1s")
                    nc.vector.tensor_tensor(out=z1[:], in0=z1_ps[:],
                                            in1=b1b_sb[:],
                                            op=mybir.AluOpType.add)
                    t02 = wk.tile([P, P], f32, tag="pr_t02")
                    nc.scalar.mul(t02[:], z1[:], NEG_SLOPE)
                    lr = wk.tile([P, P], f32, tag="pr_lr")
                    nc.vector.tensor_tensor(out=lr[:], in0=z1[:], in1=t02[:],
                                            op=mybir.AluOpType.max)
                    prod2 = wk.tile([P, P], f32, tag="pr_prod2")
                    nc.vector.tensor_tensor(out=prod2[:], in0=lr[:],
                                            in1=w2b_sb[:],
                                            op=mybir.AluOpType.mult)
                    sc_raw = sp.tile([P, 1], f32, tag="pr_scr")
                    nc.vector.tensor_reduce(out=sc_raw[:], in_=prod2[:],
                                            axis=mybir.AxisListType.X,
                                            op=mybir.AluOpType.add)
                    sc = sp.tile([P, 1], f32, tag="pr_sc")
                    nc.vector.tensor_scalar_add(sc[:], sc_raw[:],
                                                float(b2_val))
                    nc.sync.dma_start(out=dest[b_i * P:(b_i + 1) * P, :],
                                      in_=sc[:])

    _split_multi_waits(nc)
    return nc


# ---------------------------------------------------------------------------
# Entry point
# ---------------------------------------------------------------------------

_cache = {}


def kernel(**inputs):
    ins = {k: np.asarray(v) for k, v in inputs.items()}

    e0 = _dedupe_edges(ins["edge_index0"], ins["edge_weight0"])
    e1 = _dedupe_edges(ins["edge_index1"], ins["edge_weight1"])
    n_et = _max_edge_tiles(e0, e1)
    shards = [_shard_edges(*e0, n_et), _shard_edges(*e1, n_et)]

    b2_val = float(np.asarray(ins["pred_b2"]).reshape(-1)[0])

    key = (n_et, b2_val)
    if key not in _cache:
        _cache[key] = _build_program(n_et, b2_val)
    nc = _cache[key]

    def bcast(v, cols=D):
        v = np.asarray(v, np.float32).reshape(1, -1)
        return np.broadcast_to(v, (P, v.shape[1])).copy()

    common = {
        "xT": np.ascontiguousarray(ins["x"].T.astype(np.float32)),
        "ipwT": np.ascontiguousarray(
            ins["input_proj_w"].astype(np.float32).T),
        "ipb": bcast(ins["input_proj_b"]),
        "w1T": np.ascontiguousarray(ins["pred_w1"].astype(np.float32).T),
        "b1c": np.asarray(ins["pred_b1"], np.float32).reshape(P, 1).copy(),
        "w2c": np.asarray(ins["pred_w2"], np.float32).reshape(P, 1).copy(),
    }
    for l in range(2):
        in_w = ins[f"l{l}_in_w"].astype(np.float32)
        in_b = ins[f"l{l}_in_b"].astype(np.float32)
        common[f"l{l}_qwT"] = np.ascontiguousarray(in_w[0:D].T)
        common[f"l{l}_kwT"] = np.ascontiguousarray(in_w[D:2 * D].T)
        common[f"l{l}_vwT"] = np.ascontiguousarray(in_w[2 * D:3 * D].T)
        common[f"l{l}_owT"] = np.ascontiguousarray(
            ins[f"l{l}_out_w"].astype(np.float32).T)
        common[f"l{l}_fwT"] = np.ascontiguousarray(
            ins[f"l{l}_ff_w"].astype(np.float32).T)
        common[f"l{l}_qb"] = bcast(in_b[0:D])
        common[f"l{l}_kb"] = bcast(in_b[D:2 * D])
        common[f"l{l}_vb"] = bcast(in_b[2 * D:3 * D])
        common[f"l{l}_ob"] = bcast(ins[f"l{l}_out_b"])
        common[f"l{l}_fb"] = bcast(ins[f"l{l}_ff_b"])
        common[f"l{l}_n1g"] = bcast(ins[f"l{l}_n1_g"])
        common[f"l{l}_n1b"] = bcast(ins[f"l{l}_n1_b"])
        common[f"l{l}_n2g"] = bcast(ins[f"l{l}_n2_g"])
        common[f"l{l}_n2b"] = bcast(ins[f"l{l}_n2_b"])

    def tile_idx(arr_1d, c):
        a = np.asarray(arr_1d).astype(np.int32)[c * (EP // NCORES):(c + 1) * (EP // NCORES)]
        return np.ascontiguousarray(a.reshape(PB, P).T)

    in_maps = []
    for c in range(NCORES):
        m = dict(common)
        for l in range(2):
            sh = shards[l][c]
            m[f"l{l}_src"] = sh["src"]
            m[f"l{l}_dl"] = sh["dl"]
            m[f"l{l}_ew"] = sh["ew"]
        wi = np.empty((P, NDT), np.int32)
        for t in range(NDT):
            wi[:, t] = c * R + t * P + np.arange(P)
        m["winids"] = wi
        m["psrc"] = tile_idx(ins["pos_src"], c)
        m["pdst"] = tile_idx(ins["pos_dst"], c)
        m["nsrc"] = tile_idx(ins["neg_src"], c)
        m["ndst"] = tile_idx(ins["neg_dst"], c)
        in_maps.append(m)

    res = run_bass_kernel_spmd(nc, in_maps, core_ids=list(range(NCORES)))

    pos = np.concatenate([res.results[c]["pos_out"] for c in range(NCORES)], 0)
    neg = np.concatenate([res.results[c]["neg_out"] for c in range(NCORES)], 0)
    h_comb = res.results[0]["hcomb_out"]
    return pos, neg, h_comb
